# revision 1
# baseline (speedup 1.0000x reference)
"""BEV camera-to-grid scatter kernel for Trainium2 (8 NeuronCores).

Strategy:
 - Host (O(cameras) work only): compose the per-camera affine geometry into
   per-(camera, depth-slab, h-half) "unit" coefficients; compute exact f32
   cell-boundary thresholds (replicating the reference's divide+trunc binning
   bit-for-bit); conservatively cull units and bound each unit's BEV window via
   rigorous interval arithmetic; pack per-core tables.
 - Device: per core, stream only surviving feature blocks (~23% of input),
   compute per-point geometry in f32 (op-order identical to the reference
   pipeline), bin points by threshold compares + segmented scan, build per-tile
   one-hot matrices and scatter-accumulate via float32r matmuls into per-unit
   PSUM windows, accumulated into an SBUF-resident hot-region grid, then
   AllReduce partial regions across the 8 cores.
 - Host: paste the reduced hot region into the (mostly zero) full output.
"""
import sys
import numpy as np

sys.path.insert(0, '/opt/trn_rl_repo')

B, N, D, FH, FW, C = 1, 6, 118, 32, 88, 80
IH, IW = 256, 704
NX, NY, NZ = 360, 360, 1
DXS = (0.3, 0.3, 20.0)
COFF = (-54.0, -54.0, -10.0)   # bx - dx/2 per axis
NCORES = 8
HHALF = 16
UPIX = HHALF * FW          # 1408
UJ = UPIX // 128           # 11 free columns per partition
NCOEF = 21
BIGPEN = 1.0e6


def _frustum_axes():
    ds = np.arange(1.0, 60.0, 0.5, dtype=np.float32)
    xs = np.linspace(0.0, IW - 1, FW, dtype=np.float32)
    ys = np.linspace(0.0, IH - 1, FH, dtype=np.float32)
    return ds, xs, ys


def _compute_coeffs(camera2ego, lidar2ego, camera_intrinsics, img_aug_matrix, lidar_aug_matrix):
    aug = np.asarray(img_aug_matrix, np.float64)
    c2e = np.asarray(camera2ego, np.float64)
    intr = np.asarray(camera_intrinsics, np.float64)
    l2e = np.asarray(lidar2ego, np.float64)
    laug = np.asarray(lidar_aug_matrix, np.float64)
    inv_pr = np.linalg.inv(aug[..., :3, :3])
    post_trans = aug[..., :3, 3]
    A64 = inv_pr
    b64 = -np.einsum('bnij,bnj->bni', inv_pr, post_trans)
    combine = c2e[..., :3, :3] @ np.linalg.inv(intr[..., :3, :3])
    pre = laug[..., :3, :3] @ np.linalg.inv(l2e[..., :3, :3])
    M64 = np.einsum('bij,bnjk->bnik', pre, combine)
    t64 = np.einsum('bij,bnj->bni', pre, c2e[..., :3, 3] - l2e[..., :3, 3][:, None, :]) \
        + laug[..., :3, 3][:, None, :]
    return (A64[0].astype(np.float32), b64[0].astype(np.float32),
            M64[0].astype(np.float32), t64[0].astype(np.float32))


def _compute_thresholds():
    """Exact f32 thresholds replicating trunc((g - COFF)/dx) binning."""
    out = []
    for ax, nb in ((0, NX), (1, NY), (2, NZ)):
        coff = np.float32(COFF[ax]); dx = np.float32(DXS[ax])

        def q_of(g):
            return np.float32(np.float32(np.float32(g) - coff) / dx)

        def smallest(pred, lo, hi):
            def key(i):
                return np.int64(i) if i >= 0 else np.int64(-2147483648) - np.int64(i)
            def unkey(k):
                return np.int32(k) if k >= 0 else np.int32(-(k + 2147483648))
            kl = key(np.float32(lo).view(np.int32)); kh = key(np.float32(hi).view(np.int32))
            assert not pred(unkey(kl).view(np.float32)) and pred(unkey(kh).view(np.float32))
            while kh - kl > 1:
                km = (kl + kh) // 2
                if pred(unkey(km).view(np.float32)):
                    kh = km
                else:
                    kl = km
            return unkey(kh).view(np.float32)

        lo_p = np.float32(coff - 4 * dx); hi_p = np.float32(coff + (nb + 4) * dx)
        L = np.empty(nb + 1, np.float32)
        L[0] = smallest(lambda g: q_of(g) > np.float32(-1.0), lo_p, hi_p)
        for k in range(1, nb + 1):
            L[k] = smallest(lambda g, k=k: q_of(g) >= np.float32(k), lo_p, hi_p)
        out.append(L)
    return out


class _Iv:
    __slots__ = ('lo', 'hi')
    def __init__(self, lo, hi):
        self.lo = float(min(lo, hi)); self.hi = float(max(lo, hi))
    def __add__(self, o):
        if isinstance(o, _Iv):
            return _Iv(self.lo + o.lo, self.hi + o.hi)
        return _Iv(self.lo + o, self.hi + o)
    def __mul__(self, o):
        if isinstance(o, _Iv):
            c = [self.lo * o.lo, self.lo * o.hi, self.hi * o.lo, self.hi * o.hi]
            return _Iv(min(c), max(c))
        return _Iv(self.lo * o, self.hi * o) if o >= 0 else _Iv(self.hi * o, self.lo * o)
    __rmul__ = __mul__
    def intersect(self, o):
        lo = max(self.lo, o.lo); hi = min(self.hi, o.hi)
        return _Iv(lo, hi) if lo <= hi else None
    def pad(self, e):
        return _Iv(self.lo - e, self.hi + e)


def _plan_units(A, b, M, t, Lx, Ly, Lz):
    ds, xs, ys = _frustum_axes()
    EPS = 2e-3
    zlo, zhi = float(Lz[0]), float(Lz[1])
    units = []
    for n in range(N):
        An = A[n].astype(np.float64); bn = b[n].astype(np.float64)
        Mn = M[n].astype(np.float64); tn = t[n].astype(np.float64)
        for d in range(D):
            dv = float(ds[d])
            for half in range(FH // HHALF):
                pyv = ys[half * HHALF:(half + 1) * HHALF].astype(np.float64)
                pxI = _Iv(float(xs[0]), float(xs[-1]))
                pyI = _Iv(float(pyv[0]), float(pyv[-1]))
                p0 = [(An[i, 0] * pxI + An[i, 1] * pyI + (An[i, 2] * dv + bn[i])).pad(EPS)
                      for i in range(3)]
                zI = p0[2]
                qI = (Mn[2, 0] * p0[0] + Mn[2, 1] * p0[1] + Mn[2, 2]).pad(1e-6)
                gzI = (zI * qI + tn[2]).pad(EPS)
                if gzI.intersect(_Iv(zlo - EPS, zhi + EPS)) is None:
                    continue
                zc = zI
                if qI.lo > 1e-6 or qI.hi < -1e-6:
                    cands = [(zlo - EPS - tn[2]) / qI.lo, (zlo - EPS - tn[2]) / qI.hi,
                             (zhi + EPS - tn[2]) / qI.lo, (zhi + EPS - tn[2]) / qI.hi]
                    zc = zI.intersect(_Iv(min(cands), max(cands))) or zI
                rxI = (Mn[0, 0] * p0[0] + Mn[0, 1] * p0[1] + Mn[0, 2]).pad(1e-6)
                ryI = (Mn[1, 0] * p0[0] + Mn[1, 1] * p0[1] + Mn[1, 2]).pad(1e-6)
                gxI = (zc * rxI + tn[0]).pad(EPS)
                gyI = (zc * ryI + tn[1]).pad(EPS)
                kx0 = max(0, int(np.searchsorted(Lx, np.float32(gxI.lo), 'right')) - 1)
                kx1 = min(NX - 1, int(np.searchsorted(Lx, np.float32(gxI.hi), 'right')) - 1)
                ky0 = max(0, int(np.searchsorted(Ly, np.float32(gyI.lo), 'right')) - 1)
                ky1 = min(NY - 1, int(np.searchsorted(Ly, np.float32(gyI.hi), 'right')) - 1)
                if kx1 < kx0 or ky1 < ky0:
                    continue
                kx0 = max(0, kx0 - 1); kx1 = min(NX - 1, kx1 + 1)
                ky0 = max(0, ky0 - 1); ky1 = min(NY - 1, ky1 + 1)
                units.append(dict(n=n, d=d, half=half, kx0=kx0, wx=kx1 - kx0 + 1,
                                  ky0=ky0, wy=ky1 - ky0 + 1))
    return units


def _build_plan(inputs):
    A, b, M, t = _compute_coeffs(inputs['camera2ego'], inputs['lidar2ego'],
                                 inputs['camera_intrinsics'], inputs['img_aug_matrix'],
                                 inputs['lidar_aug_matrix'])
    Lx, Ly, Lz = _compute_thresholds()
    units = _plan_units(A, b, M, t, Lx, Ly, Lz)
    assert units, "no units survived culling"
    # split units whose window exceeds 1024 cells into y-subwindows; each
    # sub-unit gets a one-sided y mask at the split boundary
    split = []
    for u in units:
        parts = [dict(u, ylo=None, yhi=None)]
        while any(p['wx'] * p['wy'] > 1024 for p in parts):
            nparts = []
            for p in parts:
                if p['wx'] * p['wy'] > 1024:
                    wy1 = p['wy'] // 2
                    ysplit = float(Ly[p['ky0'] + wy1])
                    nparts.append(dict(p, wy=wy1, yhi=ysplit))
                    nparts.append(dict(p, ky0=p['ky0'] + wy1, wy=p['wy'] - wy1,
                                       ylo=ysplit))
                else:
                    nparts.append(p)
            parts = nparts
        split.extend(parts)
    units = split
    for u in units:
        assert u['wx'] * u['wy'] <= 1024, (u['wx'], u['wy'])
    rx0 = min(u['kx0'] for u in units); rx1 = max(u['kx0'] + u['wx'] for u in units)
    ry0 = min(u['ky0'] for u in units); ry1 = max(u['ky0'] + u['wy'] for u in units)
    Rx, Ry = rx1 - rx0, ry1 - ry0
    rcells = Rx * Ry

    # LPT balance across cores by approximate DVE cost
    order = sorted(range(len(units)), key=lambda i: -(units[i]['wx'] * units[i]['wy']))
    loads = [0.0] * NCORES
    percore = [[] for _ in range(NCORES)]
    for i in order:
        u = units[i]
        k = min(range(NCORES), key=lambda c: loads[c])
        percore[k].append(i)
        loads[k] += u['wx'] * u['wy'] + 2 * (u['wx'] + u['wy']) + 256
    smax = max(len(p) for p in percore)

    ds, xs, ys = _frustum_axes()
    i = np.arange(UPIX)
    pxt_flat = xs[i % FW].reshape(128, UJ)
    # py depends on unit's half
    pyt_half = [ys[h * HHALF + (i // FW)].reshape(128, UJ) for h in range(FH // HHALF)]

    thrmax = max(sum(units[i]['wx'] - 1 + units[i]['wy'] - 1 for i in pc) for pc in percore)
    thrmax = max(thrmax, 2)
    f32 = np.float32
    plan = dict(Lx=Lx, Ly=Ly, Lz=Lz, rx0=rx0, ry0=ry0, Rx=Rx, Ry=Ry, rcells=rcells,
                smax=smax, thrmax=thrmax, cores=[])
    for k in range(NCORES):
        ulist = []
        pxt = np.zeros((128, smax * UJ), np.float32)
        pyt = np.zeros((128, smax * UJ), np.float32)
        coef = np.zeros((smax, NCOEF), np.float32)
        thr = np.full((thrmax,), 3.0e38, np.float32)
        toff = 0
        for s in range(smax):
            if s < len(percore[k]):
                u = units[percore[k][s]]
                n, d, half = u['n'], u['d'], u['half']
                dv = ds[d]
                pxt[:, s * UJ:(s + 1) * UJ] = pxt_flat
                pyt[:, s * UJ:(s + 1) * UJ] = pyt_half[half]
                cc = []
                for kk in range(3):
                    c2 = f32(f32(A[n][kk, 2] * dv) + b[n][kk])
                    cc += [A[n][kk, 0], A[n][kk, 1], c2]
                for kk in range(3):
                    cc += [M[n][kk, 0], M[n][kk, 1], M[n][kk, 2], t[n][kk]]
                coef[s] = np.array(cc, np.float32)
                segx = u['wx'] - 1; segy = u['wy'] - 1
                ox, oy = toff, toff + segx
                thr[ox:ox + segx] = Lx[u['kx0'] + 1: u['kx0'] + u['wx']]
                thr[oy:oy + segy] = Ly[u['ky0'] + 1: u['ky0'] + u['wy']]
                toff += segx + segy
                ulist.append(dict(slot=s, n=n, d=d, half=half, wx=u['wx'], wy=u['wy'],
                                  kx0=u['kx0'], ky0=u['ky0'], ox=ox, oy=oy,
                                  rxo=u['kx0'] - rx0, ryo=u['ky0'] - ry0,
                                  ylo=u.get('ylo'), yhi=u.get('yhi')))
            else:
                coef[s] = 0.0
                coef[s][20] = 1.0e9   # t_z -> gz=1e9 -> masked out
                ulist.append(dict(slot=s, n=-1, d=-1, half=0, wx=2, wy=2,
                                  kx0=rx0, ky0=ry0, ox=0, oy=0, rxo=0, ryo=0,
                                  ylo=None, yhi=None))
        coef_t = np.broadcast_to(coef.reshape(1, smax * NCOEF), (128, smax * NCOEF)).copy()
        thr_t = np.broadcast_to(thr.reshape(1, thrmax), (128, thrmax)).copy()
        plan['cores'].append(dict(units=ulist, pxt=pxt, pyt=pyt, coef=coef_t, thr=thr_t,
                                  real=len(percore[k])))
    gate = np.ones((128, 128), np.float32); gate[:, 0] = 0.0
    iota = np.broadcast_to(np.arange(1024, dtype=np.float32).reshape(1, 1024),
                           (128, 1024)).copy()
    plan['gate'] = gate
    plan['iota'] = iota
    return plan


def _pack_feats(cam_feats, plan):
    """Per-core feats stack [smax, 1408, 80] from the culled half-slabs."""
    smax = plan['smax']
    outs = []
    cf = np.ascontiguousarray(np.asarray(cam_feats, np.float32)[0])  # [N,D,FH,FW,C]
    for core in plan['cores']:
        f = np.zeros((smax, UPIX, C), np.float32)
        for u in core['units']:
            if u['n'] >= 0:
                blk = cf[u['n'], u['d'], u['half'] * HHALF:(u['half'] + 1) * HHALF]
                f[u['slot']] = blk.reshape(UPIX, C)
        outs.append(f)
    return outs


_CACHE = {}


def _build_bass(plan):
    import concourse.bacc as bacc
    import concourse.mybir as mybir
    import concourse.tile as tile

    smax, thrmax, rcells = plan['smax'], plan['thrmax'], plan['rcells']
    SJ = smax * UJ
    f32, f32r = mybir.dt.float32, mybir.dt.float32r
    AL = mybir.AluOpType

    nc = bacc.Bacc(None, target_bir_lowering=False, num_devices=NCORES)
    feats_t = nc.dram_tensor("feats", [smax, UPIX, C], f32, kind="ExternalInput")
    pxt_t = nc.dram_tensor("pxt", [128, SJ], f32, kind="ExternalInput")
    pyt_t = nc.dram_tensor("pyt", [128, SJ], f32, kind="ExternalInput")
    coef_t = nc.dram_tensor("coef", [128, smax * NCOEF], f32, kind="ExternalInput")
    thr_t = nc.dram_tensor("thr", [128, thrmax], f32, kind="ExternalInput")
    gate_t = nc.dram_tensor("gate", [128, 128], f32, kind="ExternalInput")
    iota_t = nc.dram_tensor("iota", [128, 1024], f32, kind="ExternalInput")
    rout_t = nc.dram_tensor("region_out", [C, rcells], f32, kind="ExternalOutput")

    pid = nc.partition_id()
    Lz = plan['Lz']; Lx = plan['Lx']; Ly = plan['Ly']
    LZ0, LZ1 = float(Lz[0]), float(Lz[1])
    LX0, LX1 = float(Lx[0]), float(Lx[NX])
    LY0, LY1 = float(Ly[0]), float(Ly[NY])

    with tile.TileContext(nc) as tc:
        with tc.tile_pool(name="tabs", bufs=1) as tp, \
             tc.tile_pool(name="geo", bufs=1) as gp, \
             tc.tile_pool(name="work", bufs=3) as wp, \
             tc.tile_pool(name="oh", bufs=4) as op_, \
             tc.tile_pool(name="ps", bufs=3, space="PSUM") as pp, \
             tc.tile_pool(name="dram", bufs=1, space="DRAM") as dp:

            pxt = tp.tile([128, SJ], f32); nc.sync.dma_start(pxt[:], pxt_t[:])
            pyt = tp.tile([128, SJ], f32); nc.sync.dma_start(pyt[:], pyt_t[:])
            coef = tp.tile([128, smax * NCOEF], f32); nc.sync.dma_start(coef[:], coef_t[:])
            thr = tp.tile([128, thrmax], f32); nc.sync.dma_start(thr[:], thr_t[:])
            gate = tp.tile([128, 128], f32); nc.sync.dma_start(gate[:], gate_t[:])
            iota = tp.tile([128, 1024], f32); nc.sync.dma_start(iota[:], iota_t[:])
            region = gp.tile([C, rcells], f32)
            nc.vector.memset(region[:], 0.0)

            def cslice(kidx):
                # [128, smax] coefficient column kidx, broadcast over the 11 j-cols
                ap = coef[:].rearrange("p (s k) -> p s k", k=NCOEF)[:, :, kidx:kidx + 1]
                return ap.broadcast_to([128, smax, UJ])

            def g3(ap):
                return ap.rearrange("p (s j) -> p s j", j=UJ)

            # ---- batched geometry (uniform across cores; per-core data) ----
            tmpa = gp.tile([128, SJ], f32)
            tmpb = gp.tile([128, SJ], f32)
            p0 = [gp.tile([128, SJ], f32, name=f'p0_{i}', tag=f'p0_{i}') for i in range(3)]
            for kk in range(3):
                nc.vector.tensor_tensor(out=g3(tmpa[:]), in0=g3(pxt[:]), in1=cslice(3 * kk + 0), op=AL.mult)
                nc.vector.tensor_tensor(out=g3(tmpb[:]), in0=g3(pyt[:]), in1=cslice(3 * kk + 1), op=AL.mult)
                nc.vector.tensor_tensor(out=tmpa[:], in0=tmpa[:], in1=tmpb[:], op=AL.add)
                nc.vector.tensor_tensor(out=g3(p0[kk][:]), in0=g3(tmpa[:]), in1=cslice(3 * kk + 2), op=AL.add)
            uu = gp.tile([128, SJ], f32)
            vv = gp.tile([128, SJ], f32)
            nc.vector.tensor_tensor(out=uu[:], in0=p0[0][:], in1=p0[2][:], op=AL.mult)
            nc.vector.tensor_tensor(out=vv[:], in0=p0[1][:], in1=p0[2][:], op=AL.mult)
            g = [gp.tile([128, SJ], f32, name=f'g_{i}', tag=f'g_{i}') for i in range(3)]
            for kk in range(3):
                base = 9 + 4 * kk
                nc.vector.tensor_tensor(out=g3(tmpa[:]), in0=g3(uu[:]), in1=cslice(base + 0), op=AL.mult)
                nc.vector.tensor_tensor(out=g3(tmpb[:]), in0=g3(vv[:]), in1=cslice(base + 1), op=AL.mult)
                nc.vector.tensor_tensor(out=tmpa[:], in0=tmpa[:], in1=tmpb[:], op=AL.add)
                nc.vector.tensor_tensor(out=g3(tmpb[:]), in0=g3(p0[2][:]), in1=cslice(base + 2), op=AL.mult)
                nc.vector.tensor_tensor(out=tmpa[:], in0=tmpa[:], in1=tmpb[:], op=AL.add)
                nc.vector.tensor_tensor(out=g3(g[kk][:]), in0=g3(tmpa[:]), in1=cslice(base + 3), op=AL.add)
            gx, gy, gz = g
            # ---- kept mask -> additive penalty (1e6 for dropped points) ----
            pen = gp.tile([128, SJ], f32)
            nc.vector.tensor_scalar(out=tmpa[:], in0=gz[:], scalar1=LZ0, scalar2=None, op0=AL.is_ge)
            nc.vector.tensor_scalar(out=tmpb[:], in0=gz[:], scalar1=LZ1, scalar2=None, op0=AL.is_lt)
            nc.vector.tensor_tensor(out=pen[:], in0=tmpa[:], in1=tmpb[:], op=AL.mult)
            nc.vector.tensor_scalar(out=tmpa[:], in0=gx[:], scalar1=LX0, scalar2=None, op0=AL.is_ge)
            nc.vector.tensor_tensor(out=pen[:], in0=pen[:], in1=tmpa[:], op=AL.mult)
            nc.vector.tensor_scalar(out=tmpa[:], in0=gx[:], scalar1=LX1, scalar2=None, op0=AL.is_lt)
            nc.vector.tensor_tensor(out=pen[:], in0=pen[:], in1=tmpa[:], op=AL.mult)
            nc.vector.tensor_scalar(out=tmpa[:], in0=gy[:], scalar1=LY0, scalar2=None, op0=AL.is_ge)
            nc.vector.tensor_tensor(out=pen[:], in0=pen[:], in1=tmpa[:], op=AL.mult)
            nc.vector.tensor_scalar(out=tmpa[:], in0=gy[:], scalar1=LY1, scalar2=None, op0=AL.is_lt)
            nc.vector.tensor_tensor(out=pen[:], in0=pen[:], in1=tmpa[:], op=AL.mult)
            nc.vector.tensor_scalar(out=pen[:], in0=pen[:], scalar1=-BIGPEN, scalar2=BIGPEN,
                                    op0=AL.mult, op1=AL.add)

            region2d = region[:].rearrange("p (y x) -> p y x", x=plan['Rx'])

            # ---- per-core sections ----
            for core_id in range(NCORES):
                cpl = plan['cores'][core_id]
                with tc.If(pid == core_id):
                    for u in cpl['units'][:cpl['real']]:
                        s = u['slot']
                        wx, wy = u['wx'], u['wy']
                        W = wx * wy
                        ohw = W + (W & 1)
                        cols = max(256, ohw)
                        segx, segy = wx - 1, wy - 1
                        fb = wp.tile([128, UJ * C], f32, tag="fb")
                        nc.sync.dma_start(
                            fb[:], feats_t[s].rearrange("(p j) c -> p (j c)", p=128))
                        fbr = wp.tile([128, UJ * C], f32r, tag="fbr")
                        nc.gpsimd.tensor_copy(out=fbr[:], in_=fb[:])
                        # binning: compares + segmented scan (per-axis)
                        kxy = []
                        for (seg, off, gbuf) in ((segx, u['ox'], gx), (segy, u['oy'], gy)):
                            cmpb = wp.tile([128, UJ * max(seg, 1)], f32, tag="cmp")
                            scnb = wp.tile([128, UJ * max(seg, 1)], f32, tag="scn")
                            kb = wp.tile([128, UJ], f32, tag="kb", name="kb")
                            if seg > 0:
                                gsl = g3(gbuf[:])[:, s, :]
                                nc.vector.tensor_tensor(
                                    out=cmpb[:, :UJ * seg].rearrange("p (j w) -> p j w", w=seg),
                                    in0=gsl.broadcast_to([128, UJ, seg]),
                                    in1=thr[:, off:off + seg][:, None, :]
                                        .broadcast_to([128, UJ, seg]),
                                    op=AL.is_ge)
                                # plain cumsum along the whole row, then per-segment-end diffs
                                nc.vector.tensor_tensor_scan(
                                    out=scnb[:, :UJ * seg],
                                    data0=gate[:, 1:2].broadcast_to([128, UJ * seg]),
                                    data1=cmpb[:, :UJ * seg],
                                    initial=0.0, op0=AL.mult, op1=AL.add)
                                ends = scnb[:, :UJ * seg].rearrange("p (j w) -> p j w", w=seg)[:, :, seg - 1]
                                nc.vector.tensor_copy(out=kb[:, 0:1], in_=ends[:, 0:1])
                                nc.vector.tensor_tensor(out=kb[:, 1:UJ], in0=ends[:, 1:UJ],
                                                        in1=ends[:, 0:UJ - 1], op=AL.subtract)
                            else:
                                nc.vector.memset(kb[:], 0.0)
                            kxy.append(kb[:])
                        kxl, kyl = kxy
                        lidx = wp.tile([128, UJ], f32, tag="lidx")
                        nc.vector.tensor_scalar(out=lidx[:], in0=kyl, scalar1=float(wx),
                                                scalar2=None, op0=AL.mult)
                        nc.vector.tensor_tensor(out=lidx[:], in0=lidx[:], in1=kxl, op=AL.add)
                        nc.vector.tensor_tensor(
                            out=lidx[:], in0=lidx[:],
                            in1=g3(pen[:])[:, s, :], op=AL.add)
                        for bnd, opc in ((u['ylo'], AL.is_ge), (u['yhi'], AL.is_lt)):
                            if bnd is not None:
                                msk = wp.tile([128, UJ], f32, tag="msk")
                                gysl = g3(gy[:])[:, s, :]
                                nc.vector.tensor_scalar(out=msk[:], in0=gysl,
                                                        scalar1=float(bnd), scalar2=None,
                                                        op0=opc)
                                nc.vector.tensor_scalar(out=msk[:], in0=msk[:],
                                                        scalar1=-BIGPEN, scalar2=BIGPEN,
                                                        op0=AL.mult, op1=AL.add)
                                nc.vector.tensor_tensor(out=lidx[:], in0=lidx[:],
                                                        in1=msk[:], op=AL.add)
                        ps = pp.tile([C, max(cols, 512) if cols > 512 else cols],
                                     mybir.dt.float32, space="PSUM", tag="ups")
                        for j in range(UJ):
                            oh = op_.tile([128, 1024], f32r, tag="oh")
                            nc.vector.tensor_scalar(out=oh[:, :ohw], in0=iota[:, :ohw],
                                                    scalar1=lidx[:, j:j + 1], scalar2=None,
                                                    op0=AL.is_equal)
                            c1 = min(cols, 512)
                            nc.tensor.matmul(ps[:, :c1], lhsT=fbr[:, j * C:(j + 1) * C],
                                             rhs=oh[:, :c1], start=(j == 0), stop=(j == UJ - 1))
                            if cols > 512:
                                nc.tensor.matmul(ps[:, 512:cols], lhsT=fbr[:, j * C:(j + 1) * C],
                                                 rhs=oh[:, 512:cols], start=(j == 0),
                                                 stop=(j == UJ - 1))
                        dst = region2d[:, u['ryo']:u['ryo'] + wy, u['rxo']:u['rxo'] + wx]
                        nc.vector.tensor_tensor(
                            out=dst, in0=dst,
                            in1=ps[:, :W].rearrange("p (y x) -> p y x", x=wx), op=AL.add)

            # ---- epilogue: allreduce partial regions ----
            rpart = dp.tile([C, rcells], f32)
            rsum = dp.tile([C, rcells], f32)
            nc.sync.dma_start(rpart[:], region[:])
            nc.gpsimd.collective_compute(
                "AllReduce", AL.add,
                replica_groups=[list(range(NCORES))],
                ins=[rpart[:]], outs=[rsum[:]])
            nc.sync.dma_start(rout_t[:], rsum[:])

    nc.compile()
    return nc


def kernel(**inputs) -> np.ndarray:
    from concourse.bass_utils import run_bass_kernel_spmd

    plan = _build_plan(inputs)
    key = (plan['smax'], plan['thrmax'], plan['rcells'],
           tuple(tuple((u['wx'], u['wy'], u['ox'], u['oy'], u['rxo'], u['ryo'])
                       for u in c['units']) for c in plan['cores']))
    if key not in _CACHE:
        _CACHE.clear()
        _CACHE[key] = _build_bass(plan)
    nc = _CACHE[key]

    feats = _pack_feats(inputs['cam_feats'], plan)
    in_maps = []
    for k in range(NCORES):
        cpl = plan['cores'][k]
        in_maps.append(dict(feats=feats[k], pxt=cpl['pxt'], pyt=cpl['pyt'],
                            coef=cpl['coef'], thr=cpl['thr'],
                            gate=plan['gate'], iota=plan['iota']))
    r = run_bass_kernel_spmd(nc, in_maps, core_ids=list(range(NCORES)))
    region = r.results[0]['region_out']          # [C, rcells] summed over cores
    out = np.zeros((B, C, NX, NY), np.float32)
    Rx, Ry = plan['Rx'], plan['Ry']
    blk = region.reshape(C, Ry, Rx).transpose(0, 2, 1)   # -> [C, Rx(y->cx?), Ry]
    out[0, :, plan['rx0']:plan['rx0'] + Rx, plan['ry0']:plan['ry0'] + Ry] = blk
    return out



# revision 13
# speedup vs baseline: 5.8133x; 5.8133x over previous
"""BEV camera-to-grid scatter kernel for Trainium2 (8 NeuronCores).

v2 design (program-size-minimal, uniform SPMD):
 - Host (O(cameras) work): compose per-camera affine geometry into per-unit
   (camera, depth, h-half) coefficients; cull units and bound each unit's BEV
   window via interval arithmetic.  The hot BEV region is tiny (~56x28 cells);
   every unit window fits a fixed-height "stripe" of the region, and stripe
   origins quantize to a handful of classes, so the PSUM->region paste offset
   is static per class.
 - Device (ONE uniform program, ~350 instructions): batched geometry + direct
   binning (f32 divide + floor) + stripe-local scatter index with penalty
   masking for all units at once; then a hardware For_i loop over units:
   stream the unit's features (fp16), build per-point one-hot rows
   (fp16 tensor_scalar is_equal vs iota), scatter-accumulate via matmuls into
   a PSUM stripe, stage to DRAM; per-class paste loops accumulate stripes
   into an SBUF-resident region at static offsets; AllReduce the tiny region.
 - Host: paste the reduced region into the (mostly zero) full output.
"""
import sys
import numpy as np

sys.path.insert(0, '/opt/trn_rl_repo')

B, N, D, FH, FW, C = 1, 6, 118, 32, 88, 80
IH, IW = 256, 704
NX, NY, NZ = 360, 360, 1
NCORES = 8
HHALF = 16
UPIX = HHALF * FW          # 1408
UJ = UPIX // 128           # 11 free columns per partition
NCOEF = 21
NMETA = 5                  # D0, kx0, kx1, ky0, ky1
BIGPEN = 60000.0
MAGIC = np.float32(2 ** 23)

_f32 = np.float32
DXV = _f32(0.3)
DZV = _f32(20.0)
# replicate the reference's f32 constant arithmetic for (bx - dx/2)
_BX = _f32(-54.0 + 0.3 / 2.0)
COFFX = _f32(_BX - DXV / _f32(2.0))
_BZ = _f32(-10.0 + 20.0 / 2.0)
COFFZ = _f32(_BZ - DZV / _f32(2.0))


def _frustum_axes():
    ds = np.arange(1.0, 60.0, 0.5, dtype=np.float32)
    xs = np.linspace(0.0, IW - 1, FW, dtype=np.float32)
    ys = np.linspace(0.0, IH - 1, FH, dtype=np.float32)
    return ds, xs, ys


def _compute_coeffs(camera2ego, lidar2ego, camera_intrinsics, img_aug_matrix, lidar_aug_matrix):
    aug = np.asarray(img_aug_matrix, np.float64)
    c2e = np.asarray(camera2ego, np.float64)
    intr = np.asarray(camera_intrinsics, np.float64)
    l2e = np.asarray(lidar2ego, np.float64)
    laug = np.asarray(lidar_aug_matrix, np.float64)
    inv_pr = np.linalg.inv(aug[..., :3, :3])
    post_trans = aug[..., :3, 3]
    A64 = inv_pr
    b64 = -np.einsum('bnij,bnj->bni', inv_pr, post_trans)
    combine = c2e[..., :3, :3] @ np.linalg.inv(intr[..., :3, :3])
    pre = laug[..., :3, :3] @ np.linalg.inv(l2e[..., :3, :3])
    M64 = np.einsum('bij,bnjk->bnik', pre, combine)
    t64 = np.einsum('bij,bnj->bni', pre, c2e[..., :3, 3] - l2e[..., :3, 3][:, None, :]) \
        + laug[..., :3, 3][:, None, :]
    return (A64[0].astype(np.float32), b64[0].astype(np.float32),
            M64[0].astype(np.float32), t64[0].astype(np.float32))


class _Iv:
    __slots__ = ('lo', 'hi')
    def __init__(self, lo, hi):
        self.lo = float(min(lo, hi)); self.hi = float(max(lo, hi))
    def __add__(self, o):
        if isinstance(o, _Iv):
            return _Iv(self.lo + o.lo, self.hi + o.hi)
        return _Iv(self.lo + o, self.hi + o)
    def __mul__(self, o):
        if isinstance(o, _Iv):
            c = [self.lo * o.lo, self.lo * o.hi, self.hi * o.lo, self.hi * o.hi]
            return _Iv(min(c), max(c))
        return _Iv(self.lo * o, self.hi * o) if o >= 0 else _Iv(self.hi * o, self.lo * o)
    __rmul__ = __mul__
    def intersect(self, o):
        lo = max(self.lo, o.lo); hi = min(self.hi, o.hi)
        return _Iv(lo, hi) if lo <= hi else None
    def pad(self, e):
        return _Iv(self.lo - e, self.hi + e)


def _cell_of(g, coff, dx):
    return int(np.floor((np.float64(g) - np.float64(coff)) / np.float64(dx)))


def _plan_units(A, b, M, t):
    ds, xs, ys = _frustum_axes()
    EPS = 2e-3
    zlo = float(COFFZ) - float(DZV) - EPS         # qz in (-1, 1)
    zhi = float(COFFZ) + float(DZV) + EPS
    units = []
    for n in range(N):
        An = A[n].astype(np.float64); bn = b[n].astype(np.float64)
        Mn = M[n].astype(np.float64); tn = t[n].astype(np.float64)
        for d in range(D):
            dv = float(ds[d])
            for half in range(FH // HHALF):
                pyv = ys[half * HHALF:(half + 1) * HHALF].astype(np.float64)
                pxI = _Iv(float(xs[0]), float(xs[-1]))
                pyI = _Iv(float(pyv[0]), float(pyv[-1]))
                p0 = [(An[i, 0] * pxI + An[i, 1] * pyI + (An[i, 2] * dv + bn[i])).pad(EPS)
                      for i in range(3)]
                zI = p0[2]
                qI = (Mn[2, 0] * p0[0] + Mn[2, 1] * p0[1] + Mn[2, 2]).pad(1e-6)
                gzI = (zI * qI + tn[2]).pad(EPS)
                if gzI.intersect(_Iv(zlo, zhi)) is None:
                    continue
                zc = zI
                if qI.lo > 1e-6 or qI.hi < -1e-6:
                    cands = [(zlo - tn[2]) / qI.lo, (zlo - tn[2]) / qI.hi,
                             (zhi - tn[2]) / qI.lo, (zhi - tn[2]) / qI.hi]
                    zc = zI.intersect(_Iv(min(cands), max(cands))) or zI
                rxI = (Mn[0, 0] * p0[0] + Mn[0, 1] * p0[1] + Mn[0, 2]).pad(1e-6)
                ryI = (Mn[1, 0] * p0[0] + Mn[1, 1] * p0[1] + Mn[1, 2]).pad(1e-6)
                gxI = (zc * rxI + tn[0]).pad(EPS)
                gyI = (zc * ryI + tn[1]).pad(EPS)
                kx0 = max(0, _cell_of(gxI.lo, COFFX, DXV) - 1)
                kx1 = min(NX - 1, _cell_of(gxI.hi, COFFX, DXV) + 1)
                ky0 = max(0, _cell_of(gyI.lo, COFFX, DXV) - 1)
                ky1 = min(NY - 1, _cell_of(gyI.hi, COFFX, DXV) + 1)
                if kx1 < kx0 or ky1 < ky0:
                    continue
                units.append(dict(n=n, d=d, half=half, kx0=kx0, wx=kx1 - kx0 + 1,
                                  ky0=ky0, wy=ky1 - ky0 + 1))
    return units


def _build_plan(inputs):
    A, b, M, t = _compute_coeffs(inputs['camera2ego'], inputs['lidar2ego'],
                                 inputs['camera_intrinsics'], inputs['img_aug_matrix'],
                                 inputs['lidar_aug_matrix'])
    units = _plan_units(A, b, M, t)
    assert units, "no units survived culling"
    rx0 = min(u['kx0'] for u in units); rx1 = max(u['kx0'] + u['wx'] for u in units)
    ry0 = min(u['ky0'] for u in units); ry1 = max(u['ky0'] + u['wy'] for u in units)
    assert rx0 > 0 and ry0 > 0, "region touches cell 0; floor!=trunc edge unsupported"
    Rx, Ry = rx1 - rx0, ry1 - ry0
    maxwy = max(u['wy'] for u in units)
    # stripe height: smallest covering height; keep W within 3 PSUM banks
    H = min(Ry, maxwy + 1)
    W = H * Rx
    assert W <= 1536, (H, Rx)
    cap = max(Ry - H, 0)
    step = max(H - maxwy + 1, 1)
    classes = list(range(0, cap + 1, step))
    if classes[-1] != cap:
        classes.append(cap)

    # assign each unit the largest class whose stripe covers its y-window
    for u in units:
        so = None
        for s in classes:
            if s <= u['ky0'] - ry0 and (u['ky0'] - ry0) + u['wy'] <= s + H:
                so = s
        assert so is not None, (u, classes, H)
        u['cls'] = so
    used = sorted({u['cls'] for u in units})
    classes = used
    cls_index = {s: i for i, s in enumerate(classes)}

    # distribute per class round-robin across cores, pad to equal counts;
    # pad the total to a multiple of 3 (main-loop unroll) with dummy slots
    percls = [[] for _ in classes]
    for i, u in enumerate(units):
        percls[cls_index[u['cls']]].append(i)
    cls_cnt2 = [-(-len(p) // NCORES) for p in percls]
    while sum(cls_cnt2) % 3:
        cls_cnt2[-1] += 1
    smax = sum(cls_cnt2)

    ds_, xs, ys = _frustum_axes()
    i = np.arange(UPIX)
    pxt = xs[i % FW].reshape(128, UJ)
    pyt_half = [ys[h * HHALF + (i // FW)].reshape(128, UJ) for h in range(FH // HHALF)]

    plan = dict(rx0=rx0, ry0=ry0, Rx=Rx, Ry=Ry, H=H, W=W, classes=classes,
                cls_cnt=cls_cnt2, smax=smax, units=units, cores=[])
    f = np.float32
    for k in range(NCORES):
        slots = []      # (slot, unit_idx or None, class_origin)
        s = 0
        for ci, cnt in enumerate(cls_cnt2):
            mine = percls[ci][k::NCORES]
            assert len(mine) <= cnt
            mine = mine + [None] * (cnt - len(mine))
            for ui in mine:
                slots.append((s, ui, classes[ci]))
                s += 1
        assert s == smax
        pyts = np.zeros((128, smax * UJ), np.float32)
        coef = np.zeros((smax, NCOEF), np.float32)
        meta = np.zeros((smax, NMETA), np.float32)
        ulist = []
        for (s, ui, so) in slots:
            if ui is not None:
                u = units[ui]
                n, d, half = u['n'], u['d'], u['half']
                dv = ds_[d]
                pyts[:, s * UJ:(s + 1) * UJ] = pyt_half[half]
                cc = []
                for kk in range(3):
                    c2 = f(f(A[n][kk, 2] * dv) + b[n][kk])
                    cc += [A[n][kk, 0], A[n][kk, 1], c2]
                for kk in range(3):
                    cc += [M[n][kk, 0], M[n][kk, 1], M[n][kk, 2], t[n][kk]]
                coef[s] = np.array(cc, np.float32)
                meta[s] = [f((so + ry0) * Rx + rx0), f(u['kx0']), f(u['kx0'] + u['wx']),
                           f(u['ky0']), f(u['ky0'] + u['wy'])]
                ulist.append(dict(slot=s, n=n, d=d, half=half, so=so))
            else:
                coef[s] = 0.0
                coef[s][20] = 1.0e9          # t_z -> gz huge -> z-pen kills all
                meta[s] = [0.0, 0.0, 0.0, 0.0, 0.0]   # kx1==kx0 -> empty window
                ulist.append(dict(slot=s, n=-1, d=-1, half=0, so=so))
        coef_t = np.broadcast_to(coef.reshape(1, smax * NCOEF), (128, smax * NCOEF)).copy()
        meta_t = np.broadcast_to(meta.reshape(1, smax * NMETA), (128, smax * NMETA)).copy()
        plan['cores'].append(dict(units=ulist, pyt=pyts, coef=coef_t, meta=meta_t))
    plan['pxt'] = np.ascontiguousarray(pxt)
    return plan


def _pack_feats(cam_feats, plan):
    """Per-core feats stack [smax, 128, UJ*C] fp16 from the culled half-slabs."""
    smax = plan['smax']
    outs = []
    cf = np.asarray(cam_feats, np.float32)[0]   # [N,D,FH,FW,C]
    for core in plan['cores']:
        fbuf = np.zeros((smax, 128, UJ * C), np.float16)
        for u in core['units']:
            if u['n'] >= 0:
                blk = cf[u['n'], u['d'], u['half'] * HHALF:(u['half'] + 1) * HHALF]
                fbuf[u['slot']] = blk.reshape(128, UJ * C)
        outs.append(fbuf)
    return outs


_CACHE = {}


def _build_bass(plan):
    import concourse.bacc as bacc
    import concourse.mybir as mybir
    import concourse.tile as tile
    from concourse.bass import ds as dslice

    smax, W, H, Rx, Ry = plan['smax'], plan['W'], plan['H'], plan['Rx'], plan['Ry']
    SJ = smax * UJ
    rcells = Rx * Ry
    f32, f16, i32 = mybir.dt.float32, mybir.dt.float16, mybir.dt.int32
    f32r = mybir.dt.float32r
    AL = mybir.AluOpType

    nc = bacc.Bacc(None, target_bir_lowering=False, num_devices=NCORES)
    feats_t = nc.dram_tensor("feats", [smax, 128, UJ * C], f16, kind="ExternalInput")
    pxt_t = nc.dram_tensor("pxt", [128, UJ], f32, kind="ExternalInput")
    pyt_t = nc.dram_tensor("pyt", [128, SJ], f32, kind="ExternalInput")
    coef_t = nc.dram_tensor("coef", [128, smax * NCOEF], f32, kind="ExternalInput")
    meta_t = nc.dram_tensor("meta", [128, smax * NMETA], f32, kind="ExternalInput")
    rout_t = nc.dram_tensor("region_out", [C, rcells], f32, kind="ExternalOutput")

    # matmul bank slices within the stripe (PSUM bank = 512 f32)
    bank_slices = [(a, min(a + 512, W)) for a in range(0, W, 512)]

    with tile.TileContext(nc) as tc:
        with tc.tile_pool(name="tabs", bufs=1) as tp, \
             tc.tile_pool(name="geo", bufs=1) as gp, \
             tc.tile_pool(name="work", bufs=3) as wp, \
             tc.tile_pool(name="oh", bufs=2) as op_, \
             tc.tile_pool(name="paste", bufs=3) as qp, \
             tc.tile_pool(name="ps", bufs=2, space="PSUM") as pp, \
             tc.tile_pool(name="dram", bufs=1, space="DRAM") as dp:

            pxt = tp.tile([128, UJ], f32); nc.sync.dma_start(pxt[:], pxt_t[:])
            pyt = tp.tile([128, SJ], f32); nc.sync.dma_start(pyt[:], pyt_t[:])
            coef = tp.tile([128, smax * NCOEF], f32); nc.sync.dma_start(coef[:], coef_t[:])
            meta = tp.tile([128, smax * NMETA], f32); nc.sync.dma_start(meta[:], meta_t[:])
            iota32 = tp.tile([128, W], i32)
            nc.gpsimd.iota(iota32[:], pattern=[[1, W]], base=0, channel_multiplier=0)
            iotaf = tp.tile([128, W], f32)
            nc.vector.tensor_copy(out=iotaf[:], in_=iota32[:])
            region = gp.tile([C, rcells], f32)
            nc.vector.memset(region[:], 0.0)

            def cslice(kidx):
                ap = coef[:].rearrange("p (s k) -> p s k", k=NCOEF)[:, :, kidx:kidx + 1]
                return ap.broadcast_to([128, smax, UJ])

            def mslice(kidx):
                ap = meta[:].rearrange("p (s k) -> p s k", k=NMETA)[:, :, kidx:kidx + 1]
                return ap.broadcast_to([128, smax, UJ])

            def g3(ap):
                return ap.rearrange("p (s j) -> p s j", j=UJ)

            # ---- batched geometry (identical op order to the reference) ----
            tmpa = gp.tile([128, SJ], f32)
            tmpb = gp.tile([128, SJ], f32)
            pxb = pxt[:][:, None, :].broadcast_to([128, smax, UJ])
            p0 = [gp.tile([128, SJ], f32, name=f'p0_{i}', tag=f'p0_{i}') for i in range(3)]
            for kk in range(3):
                nc.vector.tensor_tensor(out=g3(tmpa[:]), in0=pxb, in1=cslice(3 * kk + 0), op=AL.mult)
                nc.vector.tensor_tensor(out=g3(tmpb[:]), in0=g3(pyt[:]), in1=cslice(3 * kk + 1), op=AL.mult)
                nc.vector.tensor_tensor(out=tmpa[:], in0=tmpa[:], in1=tmpb[:], op=AL.add)
                nc.vector.tensor_tensor(out=g3(p0[kk][:]), in0=g3(tmpa[:]), in1=cslice(3 * kk + 2), op=AL.add)
            uu = gp.tile([128, SJ], f32)
            vv = gp.tile([128, SJ], f32)
            nc.vector.tensor_tensor(out=uu[:], in0=p0[0][:], in1=p0[2][:], op=AL.mult)
            nc.vector.tensor_tensor(out=vv[:], in0=p0[1][:], in1=p0[2][:], op=AL.mult)
            g = [gp.tile([128, SJ], f32, name=f'g_{i}', tag=f'g_{i}') for i in range(3)]
            for kk in range(3):
                base = 9 + 4 * kk
                nc.vector.tensor_tensor(out=g3(tmpa[:]), in0=g3(uu[:]), in1=cslice(base + 0), op=AL.mult)
                nc.vector.tensor_tensor(out=g3(tmpb[:]), in0=g3(vv[:]), in1=cslice(base + 1), op=AL.mult)
                nc.vector.tensor_tensor(out=tmpa[:], in0=tmpa[:], in1=tmpb[:], op=AL.add)
                nc.vector.tensor_tensor(out=g3(tmpb[:]), in0=g3(p0[2][:]), in1=cslice(base + 2), op=AL.mult)
                nc.vector.tensor_tensor(out=tmpa[:], in0=tmpa[:], in1=tmpb[:], op=AL.add)
                nc.vector.tensor_tensor(out=g3(g[kk][:]), in0=g3(tmpa[:]), in1=cslice(base + 3), op=AL.add)
            gx, gy, gz = g

            # ---- direct binning: q = (g - coff) / dx ; k = floor(q) ----
            qx = uu; qy = vv; qz = p0[0]        # reuse buffers
            invx = float(np.float32(1.0) / DXV)
            invz = float(np.float32(1.0) / DZV)
            nc.vector.tensor_scalar(out=qx[:], in0=gx[:], scalar1=float(COFFX),
                                    scalar2=invx, op0=AL.subtract, op1=AL.mult)
            nc.vector.tensor_scalar(out=qy[:], in0=gy[:], scalar1=float(COFFX),
                                    scalar2=invx, op0=AL.subtract, op1=AL.mult)
            nc.vector.tensor_scalar(out=qz[:], in0=gz[:], scalar1=float(COFFZ),
                                    scalar2=invz, op0=AL.subtract, op1=AL.mult)
            kxt = p0[1]; kyt = p0[2]
            # round-to-nearest then subtract (q < r) -> floor
            nc.vector.tensor_scalar(out=kxt[:], in0=qx[:], scalar1=float(MAGIC),
                                    scalar2=float(MAGIC), op0=AL.add, op1=AL.subtract)
            nc.vector.tensor_tensor(out=tmpa[:], in0=qx[:], in1=kxt[:], op=AL.is_lt)
            nc.vector.tensor_tensor(out=kxt[:], in0=kxt[:], in1=tmpa[:], op=AL.subtract)
            nc.vector.tensor_scalar(out=kyt[:], in0=qy[:], scalar1=float(MAGIC),
                                    scalar2=float(MAGIC), op0=AL.add, op1=AL.subtract)
            nc.vector.tensor_tensor(out=tmpa[:], in0=qy[:], in1=kyt[:], op=AL.is_lt)
            nc.vector.tensor_tensor(out=kyt[:], in0=kyt[:], in1=tmpa[:], op=AL.subtract)

            # ---- penalties: window containment + z in (-1, 1) ----
            pen = gx      # reuse
            nc.vector.tensor_tensor(out=g3(pen[:]), in0=g3(kxt[:]), in1=mslice(1), op=AL.is_ge)
            nc.vector.tensor_tensor(out=g3(tmpa[:]), in0=g3(kxt[:]), in1=mslice(2), op=AL.is_lt)
            nc.vector.tensor_tensor(out=pen[:], in0=pen[:], in1=tmpa[:], op=AL.mult)
            nc.vector.tensor_tensor(out=g3(tmpa[:]), in0=g3(kyt[:]), in1=mslice(3), op=AL.is_ge)
            nc.vector.tensor_tensor(out=pen[:], in0=pen[:], in1=tmpa[:], op=AL.mult)
            nc.vector.tensor_tensor(out=g3(tmpa[:]), in0=g3(kyt[:]), in1=mslice(4), op=AL.is_lt)
            nc.vector.tensor_tensor(out=pen[:], in0=pen[:], in1=tmpa[:], op=AL.mult)
            nc.vector.tensor_scalar(out=tmpa[:], in0=qz[:], scalar1=-1.0, scalar2=None, op0=AL.is_gt)
            nc.vector.tensor_tensor(out=pen[:], in0=pen[:], in1=tmpa[:], op=AL.mult)
            nc.vector.tensor_scalar(out=tmpa[:], in0=qz[:], scalar1=1.0, scalar2=None, op0=AL.is_lt)
            nc.vector.tensor_tensor(out=pen[:], in0=pen[:], in1=tmpa[:], op=AL.mult)

            # ---- stripe-local index: kyt*Rx + kxt - D0, clamp, apply penalty ----
            lidx = gy     # reuse
            nc.vector.tensor_scalar(out=lidx[:], in0=kyt[:], scalar1=float(Rx),
                                    scalar2=None, op0=AL.mult)
            nc.vector.tensor_tensor(out=lidx[:], in0=lidx[:], in1=kxt[:], op=AL.add)
            nc.vector.tensor_tensor(out=g3(lidx[:]), in0=g3(lidx[:]), in1=mslice(0), op=AL.subtract)
            nc.vector.tensor_scalar(out=lidx[:], in0=lidx[:], scalar1=-1000.0,
                                    scalar2=40000.0, op0=AL.max, op1=AL.min)
            nc.vector.tensor_scalar(out=tmpa[:], in0=pen[:], scalar1=-BIGPEN,
                                    scalar2=BIGPEN, op0=AL.mult, op1=AL.add)
            nc.vector.tensor_tensor(out=lidx[:], in0=lidx[:], in1=tmpa[:], op=AL.add)

            staged = dp.tile([smax, C, W], f32)
            region2d = region[:].rearrange("p (y x) -> p y x", x=Rx)

            # ---- main unit loop ----
            def body(iv):
                cur = wp.tile([128, UJ], f32, tag="cur")
                nc.sync.dma_start(cur[:], lidx[:, dslice(iv * UJ, UJ)])
                fb = wp.tile([128, UJ * C], f16, tag="fb")
                nc.sync.dma_start(fb[:], feats_t[dslice(iv, 1)].rearrange("o p x -> p (o x)"))
                fbr = wp.tile([128, UJ * C], f32r, tag="fbr")
                nc.gpsimd.tensor_copy(out=fbr[:], in_=fb[:])
                oh = op_.tile([128, UJ * W], f32r, tag="oh")
                for j in range(UJ):
                    nc.vector.tensor_scalar(out=oh[:, j * W:(j + 1) * W], in0=iotaf[:],
                                            scalar1=cur[:, j:j + 1], scalar2=None,
                                            op0=AL.is_equal)
                ps = pp.tile([C, W], f32, space="PSUM", tag="ps")
                for j in range(UJ):
                    for (a, bnd) in bank_slices:
                        nc.tensor.matmul(ps[:, a:bnd], lhsT=fbr[:, j * C:(j + 1) * C],
                                         rhs=oh[:, j * W + a:j * W + bnd],
                                         start=(j == 0), stop=(j == UJ - 1))
                pss = wp.tile([C, W], f32, tag="pss")
                nc.scalar.copy(out=pss[:], in_=ps[:])
                nc.sync.dma_start(staged[dslice(iv, 1)].rearrange("o p x -> p (o x)"), pss[:])

            tc.For_i_unrolled(0, smax, 1, body, max_unroll=3)

            # ---- per-class paste loops (static stripe offsets) ----
            off = 0
            for so, cnt in zip(plan['classes'], plan['cls_cnt']):
                def paste(iv, so=so):
                    tmp = qp.tile([C, W], f32, tag="pt")
                    nc.sync.dma_start(tmp[:], staged[dslice(iv, 1)].rearrange("o p x -> p (o x)"))
                    dst = region2d[:, so:so + H, :]
                    nc.vector.tensor_tensor(
                        out=dst, in0=dst,
                        in1=tmp[:].rearrange("p (y x) -> p y x", x=Rx), op=AL.add)
                tc.For_i_unrolled(off, off + cnt, 1, paste, max_unroll=2)
                off += cnt

            # ---- epilogue: allreduce the region ----
            rpart = dp.tile([C, rcells], f32)
            rsum = dp.tile([C, rcells], f32)
            nc.sync.dma_start(rpart[:], region[:])
            nc.gpsimd.collective_compute(
                "AllReduce", AL.add,
                replica_groups=[list(range(NCORES))],
                ins=[rpart[:]], outs=[rsum[:]])
            nc.sync.dma_start(rout_t[:], rsum[:])

    nc.compile()
    return nc


def kernel(**inputs) -> np.ndarray:
    from concourse.bass_utils import run_bass_kernel_spmd

    plan = _build_plan(inputs)
    key = (plan['smax'], plan['W'], plan['H'], plan['Rx'], plan['Ry'],
           tuple(plan['classes']), tuple(plan['cls_cnt']))
    if key not in _CACHE:
        _CACHE.clear()
        _CACHE[key] = _build_bass(plan)
    nc = _CACHE[key]

    feats = _pack_feats(inputs['cam_feats'], plan)
    in_maps = []
    for k in range(NCORES):
        cpl = plan['cores'][k]
        in_maps.append(dict(feats=feats[k], pxt=plan['pxt'], pyt=cpl['pyt'],
                            coef=cpl['coef'], meta=cpl['meta']))
    r = run_bass_kernel_spmd(nc, in_maps, core_ids=list(range(NCORES)))
    region = r.results[0]['region_out']          # [C, rcells] summed over cores
    out = np.zeros((B, C, NX, NY), np.float32)
    Rx, Ry = plan['Rx'], plan['Ry']
    blk = region.reshape(C, Ry, Rx).transpose(0, 2, 1)
    out[0, :, plan['rx0']:plan['rx0'] + Rx, plan['ry0']:plan['ry0'] + Ry] = blk
    return out


# revision 17
# speedup vs baseline: 43.0625x; 7.4076x over previous
"""BEV camera-to-grid scatter kernel for Trainium2 (8 NeuronCores).

v2 design (program-size-minimal, uniform SPMD):
 - Host (O(cameras) work): compose per-camera affine geometry into per-unit
   (camera, depth, h-half) coefficients; cull units and bound each unit's BEV
   window via interval arithmetic.  The hot BEV region is tiny (~56x28 cells);
   every unit window fits a fixed-height "stripe" of the region, and stripe
   origins quantize to a handful of classes, so the PSUM->region paste offset
   is static per class.
 - Device (ONE uniform program, ~350 instructions): batched geometry + direct
   binning (f32 divide + floor) + stripe-local scatter index with penalty
   masking for all units at once; then a hardware For_i loop over units:
   stream the unit's features (fp16), build per-point one-hot rows
   (fp16 tensor_scalar is_equal vs iota), scatter-accumulate via matmuls into
   a PSUM stripe, stage to DRAM; per-class paste loops accumulate stripes
   into an SBUF-resident region at static offsets; AllReduce the tiny region.
 - Host: paste the reduced region into the (mostly zero) full output.
"""
import sys
import numpy as np

sys.path.insert(0, '/opt/trn_rl_repo')

B, N, D, FH, FW, C = 1, 6, 118, 32, 88, 80
IH, IW = 256, 704
NX, NY, NZ = 360, 360, 1
NCORES = 8
HHALF = 16
UPIX = HHALF * FW          # 1408
UJ = UPIX // 128           # 11 free columns per partition
NCOEF = 21
NMETA = 5                  # D0, kx0, kx1, ky0, ky1
BIGPEN = 60000.0
MAGIC = np.float32(2 ** 23)

_f32 = np.float32
DXV = _f32(0.3)
DZV = _f32(20.0)
# replicate the reference's f32 constant arithmetic for (bx - dx/2)
_BX = _f32(-54.0 + 0.3 / 2.0)
COFFX = _f32(_BX - DXV / _f32(2.0))
_BZ = _f32(-10.0 + 20.0 / 2.0)
COFFZ = _f32(_BZ - DZV / _f32(2.0))


def _frustum_axes():
    ds = np.arange(1.0, 60.0, 0.5, dtype=np.float32)
    xs = np.linspace(0.0, IW - 1, FW, dtype=np.float32)
    ys = np.linspace(0.0, IH - 1, FH, dtype=np.float32)
    return ds, xs, ys


def _compute_coeffs(camera2ego, lidar2ego, camera_intrinsics, img_aug_matrix, lidar_aug_matrix):
    aug = np.asarray(img_aug_matrix, np.float64)
    c2e = np.asarray(camera2ego, np.float64)
    intr = np.asarray(camera_intrinsics, np.float64)
    l2e = np.asarray(lidar2ego, np.float64)
    laug = np.asarray(lidar_aug_matrix, np.float64)
    inv_pr = np.linalg.inv(aug[..., :3, :3])
    post_trans = aug[..., :3, 3]
    A64 = inv_pr
    b64 = -np.einsum('bnij,bnj->bni', inv_pr, post_trans)
    combine = c2e[..., :3, :3] @ np.linalg.inv(intr[..., :3, :3])
    pre = laug[..., :3, :3] @ np.linalg.inv(l2e[..., :3, :3])
    M64 = np.einsum('bij,bnjk->bnik', pre, combine)
    t64 = np.einsum('bij,bnj->bni', pre, c2e[..., :3, 3] - l2e[..., :3, 3][:, None, :]) \
        + laug[..., :3, 3][:, None, :]
    return (A64[0].astype(np.float32), b64[0].astype(np.float32),
            M64[0].astype(np.float32), t64[0].astype(np.float32))


class _Iv:
    __slots__ = ('lo', 'hi')
    def __init__(self, lo, hi):
        self.lo = float(min(lo, hi)); self.hi = float(max(lo, hi))
    def __add__(self, o):
        if isinstance(o, _Iv):
            return _Iv(self.lo + o.lo, self.hi + o.hi)
        return _Iv(self.lo + o, self.hi + o)
    def __mul__(self, o):
        if isinstance(o, _Iv):
            c = [self.lo * o.lo, self.lo * o.hi, self.hi * o.lo, self.hi * o.hi]
            return _Iv(min(c), max(c))
        return _Iv(self.lo * o, self.hi * o) if o >= 0 else _Iv(self.hi * o, self.lo * o)
    __rmul__ = __mul__
    def intersect(self, o):
        lo = max(self.lo, o.lo); hi = min(self.hi, o.hi)
        return _Iv(lo, hi) if lo <= hi else None
    def pad(self, e):
        return _Iv(self.lo - e, self.hi + e)


def _cell_of(g, coff, dx):
    return int(np.floor((np.float64(g) - np.float64(coff)) / np.float64(dx)))


def _plan_units(A, b, M, t):
    ds, xs, ys = _frustum_axes()
    EPS = 2e-3
    zlo = float(COFFZ) - float(DZV) - EPS         # qz in (-1, 1)
    zhi = float(COFFZ) + float(DZV) + EPS
    units = []
    for n in range(N):
        An = A[n].astype(np.float64); bn = b[n].astype(np.float64)
        Mn = M[n].astype(np.float64); tn = t[n].astype(np.float64)
        for d in range(D):
            dv = float(ds[d])
            for half in range(FH // HHALF):
                pyv = ys[half * HHALF:(half + 1) * HHALF].astype(np.float64)
                pxI = _Iv(float(xs[0]), float(xs[-1]))
                pyI = _Iv(float(pyv[0]), float(pyv[-1]))
                p0 = [(An[i, 0] * pxI + An[i, 1] * pyI + (An[i, 2] * dv + bn[i])).pad(EPS)
                      for i in range(3)]
                zI = p0[2]
                qI = (Mn[2, 0] * p0[0] + Mn[2, 1] * p0[1] + Mn[2, 2]).pad(1e-6)
                gzI = (zI * qI + tn[2]).pad(EPS)
                if gzI.intersect(_Iv(zlo, zhi)) is None:
                    continue
                zc = zI
                if qI.lo > 1e-6 or qI.hi < -1e-6:
                    cands = [(zlo - tn[2]) / qI.lo, (zlo - tn[2]) / qI.hi,
                             (zhi - tn[2]) / qI.lo, (zhi - tn[2]) / qI.hi]
                    zc = zI.intersect(_Iv(min(cands), max(cands))) or zI
                rxI = (Mn[0, 0] * p0[0] + Mn[0, 1] * p0[1] + Mn[0, 2]).pad(1e-6)
                ryI = (Mn[1, 0] * p0[0] + Mn[1, 1] * p0[1] + Mn[1, 2]).pad(1e-6)
                gxI = (zc * rxI + tn[0]).pad(EPS)
                gyI = (zc * ryI + tn[1]).pad(EPS)
                kx0 = max(0, _cell_of(gxI.lo, COFFX, DXV) - 1)
                kx1 = min(NX - 1, _cell_of(gxI.hi, COFFX, DXV) + 1)
                ky0 = max(0, _cell_of(gyI.lo, COFFX, DXV) - 1)
                ky1 = min(NY - 1, _cell_of(gyI.hi, COFFX, DXV) + 1)
                if kx1 < kx0 or ky1 < ky0:
                    continue
                units.append(dict(n=n, d=d, half=half, kx0=kx0, wx=kx1 - kx0 + 1,
                                  ky0=ky0, wy=ky1 - ky0 + 1))
    return units


def _build_plan(inputs):
    A, b, M, t = _compute_coeffs(inputs['camera2ego'], inputs['lidar2ego'],
                                 inputs['camera_intrinsics'], inputs['img_aug_matrix'],
                                 inputs['lidar_aug_matrix'])
    units = _plan_units(A, b, M, t)
    assert units, "no units survived culling"
    rx0 = min(u['kx0'] for u in units); rx1 = max(u['kx0'] + u['wx'] for u in units)
    ry0 = min(u['ky0'] for u in units); ry1 = max(u['ky0'] + u['wy'] for u in units)
    assert rx0 > 0 and ry0 > 0, "region touches cell 0; floor!=trunc edge unsupported"
    Rx, Ry = rx1 - rx0, ry1 - ry0
    maxwy = max(u['wy'] for u in units)
    # stripe height: smallest covering height; keep W within 3 PSUM banks
    H = min(Ry, maxwy + 1)
    W = H * Rx
    assert W <= 1536, (H, Rx)
    cap = max(Ry - H, 0)
    step = max(H - maxwy + 1, 1)
    classes = list(range(0, cap + 1, step))
    if classes[-1] != cap:
        classes.append(cap)

    # assign each unit the largest class whose stripe covers its y-window
    for u in units:
        so = None
        for s in classes:
            if s <= u['ky0'] - ry0 and (u['ky0'] - ry0) + u['wy'] <= s + H:
                so = s
        assert so is not None, (u, classes, H)
        u['cls'] = so
    used = sorted({u['cls'] for u in units})
    classes = used
    cls_index = {s: i for i, s in enumerate(classes)}

    # distribute per class round-robin across cores, pad to equal counts;
    # pad the total to a multiple of 3 (main-loop unroll) with dummy slots
    percls = [[] for _ in classes]
    for i, u in enumerate(units):
        percls[cls_index[u['cls']]].append(i)
    cls_cnt2 = [-(-len(p) // NCORES) for p in percls]
    while sum(cls_cnt2) % 3:
        cls_cnt2[-1] += 1
    smax = sum(cls_cnt2)

    ds_, xs, ys = _frustum_axes()
    i = np.arange(UPIX)
    pxt = xs[i % FW].reshape(128, UJ)
    pyt_half = [ys[h * HHALF + (i // FW)].reshape(128, UJ) for h in range(FH // HHALF)]

    plan = dict(rx0=rx0, ry0=ry0, Rx=Rx, Ry=Ry, H=H, W=W, classes=classes,
                cls_cnt=cls_cnt2, smax=smax, units=units, cores=[])
    f = np.float32
    for k in range(NCORES):
        slots = []      # (slot, unit_idx or None, class_origin)
        s = 0
        for ci, cnt in enumerate(cls_cnt2):
            mine = percls[ci][k::NCORES]
            assert len(mine) <= cnt
            mine = mine + [None] * (cnt - len(mine))
            for ui in mine:
                slots.append((s, ui, classes[ci]))
                s += 1
        assert s == smax
        pyts = np.zeros((128, smax * UJ), np.float32)
        coef = np.zeros((smax, NCOEF), np.float32)
        meta = np.zeros((smax, NMETA), np.float32)
        ulist = []
        for (s, ui, so) in slots:
            if ui is not None:
                u = units[ui]
                n, d, half = u['n'], u['d'], u['half']
                dv = ds_[d]
                pyts[:, s * UJ:(s + 1) * UJ] = pyt_half[half]
                cc = []
                for kk in range(3):
                    c2 = f(f(A[n][kk, 2] * dv) + b[n][kk])
                    cc += [A[n][kk, 0], A[n][kk, 1], c2]
                for kk in range(3):
                    cc += [M[n][kk, 0], M[n][kk, 1], M[n][kk, 2], t[n][kk]]
                coef[s] = np.array(cc, np.float32)
                meta[s] = [f((so + ry0) * Rx + rx0), f(u['kx0']), f(u['kx0'] + u['wx']),
                           f(u['ky0']), f(u['ky0'] + u['wy'])]
                ulist.append(dict(slot=s, n=n, d=d, half=half, so=so))
            else:
                coef[s] = 0.0
                coef[s][20] = 1.0e9          # t_z -> gz huge -> z-pen kills all
                meta[s] = [0.0, 0.0, 0.0, 0.0, 0.0]   # kx1==kx0 -> empty window
                ulist.append(dict(slot=s, n=-1, d=-1, half=0, so=so))
        coef_t = np.broadcast_to(coef.reshape(1, smax * NCOEF), (128, smax * NCOEF)).copy()
        meta_t = np.broadcast_to(meta.reshape(1, smax * NMETA), (128, smax * NMETA)).copy()
        plan['cores'].append(dict(units=ulist, pyt=pyts, coef=coef_t, meta=meta_t))
    plan['pxt'] = np.ascontiguousarray(pxt)
    return plan


def _pack_feats(cam_feats, plan):
    """Per-core feats stack [smax, 128, UJ*C] fp16 from the culled half-slabs."""
    smax = plan['smax']
    outs = []
    cf = np.asarray(cam_feats, np.float32)[0]   # [N,D,FH,FW,C]
    for core in plan['cores']:
        fbuf = np.zeros((smax, 128, UJ * C), np.float16)
        for u in core['units']:
            if u['n'] >= 0:
                blk = cf[u['n'], u['d'], u['half'] * HHALF:(u['half'] + 1) * HHALF]
                fbuf[u['slot']] = blk.reshape(128, UJ * C)
        outs.append(fbuf)
    return outs


_CACHE = {}


def _build_bass(plan):
    import concourse.bacc as bacc
    import concourse.mybir as mybir
    import concourse.tile as tile
    from concourse.bass import ds as dslice

    smax, W, H, Rx, Ry = plan['smax'], plan['W'], plan['H'], plan['Rx'], plan['Ry']
    SJ = smax * UJ
    rcells = Rx * Ry
    f32, f16, i32 = mybir.dt.float32, mybir.dt.float16, mybir.dt.int32
    f32r = mybir.dt.float32r
    AL = mybir.AluOpType

    nc = bacc.Bacc(None, target_bir_lowering=False, num_devices=NCORES)
    feats_t = nc.dram_tensor("feats", [smax, 128, UJ * C], f16, kind="ExternalInput")
    pxt_t = nc.dram_tensor("pxt", [128, UJ], f32, kind="ExternalInput")
    pyt_t = nc.dram_tensor("pyt", [128, SJ], f32, kind="ExternalInput")
    coef_t = nc.dram_tensor("coef", [128, smax * NCOEF], f32, kind="ExternalInput")
    meta_t = nc.dram_tensor("meta", [128, smax * NMETA], f32, kind="ExternalInput")
    rout_t = nc.dram_tensor("region_out", [C, rcells], f32, kind="ExternalOutput")

    # matmul bank slices within the stripe (PSUM bank = 512 f32)
    bank_slices = [(a, min(a + 512, W)) for a in range(0, W, 512)]

    with tile.TileContext(nc) as tc:
        with tc.tile_pool(name="tabs", bufs=1) as tp, \
             tc.tile_pool(name="geo", bufs=1) as gp, \
             tc.tile_pool(name="work", bufs=3) as wp, \
             tc.tile_pool(name="oh", bufs=1) as op_, \
             tc.tile_pool(name="ps", bufs=2, space="PSUM") as pp, \
             tc.tile_pool(name="dram", bufs=1, space="DRAM") as dp:

            pxt = tp.tile([128, UJ], f32); nc.sync.dma_start(pxt[:], pxt_t[:])
            pyt = tp.tile([128, SJ], f32); nc.sync.dma_start(pyt[:], pyt_t[:])
            coef = tp.tile([128, smax * NCOEF], f32); nc.sync.dma_start(coef[:], coef_t[:])
            meta = tp.tile([128, smax * NMETA], f32); nc.sync.dma_start(meta[:], meta_t[:])
            iota32 = gp.tile([128, W], i32)
            nc.gpsimd.iota(iota32[:], pattern=[[1, W]], base=0, channel_multiplier=0)
            iotaf = tp.tile([128, W], f32)
            nc.vector.tensor_copy(out=iotaf[:], in_=iota32[:])
            region = tp.tile([C, rcells], f32)
            nc.vector.memset(region[:], 0.0)
            regionb = tp.tile([C, rcells], f32)
            nc.gpsimd.memset(regionb[:], 0.0)
            stag = tp.tile([C, smax * W], f16)
            lidxT = tp.tile([128, SJ], f32)

            def cslice(kidx):
                ap = coef[:].rearrange("p (s k) -> p s k", k=NCOEF)[:, :, kidx:kidx + 1]
                return ap.broadcast_to([128, smax, UJ])

            def mslice(kidx):
                ap = meta[:].rearrange("p (s k) -> p s k", k=NMETA)[:, :, kidx:kidx + 1]
                return ap.broadcast_to([128, smax, UJ])

            def g3(ap):
                return ap.rearrange("p (s j) -> p s j", j=UJ)

            # ---- batched geometry (identical op order to the reference) ----
            tmpa = gp.tile([128, SJ], f32)
            tmpb = gp.tile([128, SJ], f32)
            pxb = pxt[:][:, None, :].broadcast_to([128, smax, UJ])
            p0 = [gp.tile([128, SJ], f32, name=f'p0_{i}', tag=f'p0_{i}') for i in range(3)]
            for kk in range(3):
                nc.vector.tensor_tensor(out=g3(tmpa[:]), in0=pxb, in1=cslice(3 * kk + 0), op=AL.mult)
                nc.vector.tensor_tensor(out=g3(tmpb[:]), in0=g3(pyt[:]), in1=cslice(3 * kk + 1), op=AL.mult)
                nc.vector.tensor_tensor(out=tmpa[:], in0=tmpa[:], in1=tmpb[:], op=AL.add)
                nc.vector.tensor_tensor(out=g3(p0[kk][:]), in0=g3(tmpa[:]), in1=cslice(3 * kk + 2), op=AL.add)
            uu = gp.tile([128, SJ], f32)
            vv = gp.tile([128, SJ], f32)
            nc.vector.tensor_tensor(out=uu[:], in0=p0[0][:], in1=p0[2][:], op=AL.mult)
            nc.vector.tensor_tensor(out=vv[:], in0=p0[1][:], in1=p0[2][:], op=AL.mult)
            g = [gp.tile([128, SJ], f32, name=f'g_{i}', tag=f'g_{i}') for i in range(3)]
            for kk in range(3):
                base = 9 + 4 * kk
                nc.vector.tensor_tensor(out=g3(tmpa[:]), in0=g3(uu[:]), in1=cslice(base + 0), op=AL.mult)
                nc.vector.tensor_tensor(out=g3(tmpb[:]), in0=g3(vv[:]), in1=cslice(base + 1), op=AL.mult)
                nc.vector.tensor_tensor(out=tmpa[:], in0=tmpa[:], in1=tmpb[:], op=AL.add)
                nc.vector.tensor_tensor(out=g3(tmpb[:]), in0=g3(p0[2][:]), in1=cslice(base + 2), op=AL.mult)
                nc.vector.tensor_tensor(out=tmpa[:], in0=tmpa[:], in1=tmpb[:], op=AL.add)
                nc.vector.tensor_tensor(out=g3(g[kk][:]), in0=g3(tmpa[:]), in1=cslice(base + 3), op=AL.add)
            gx, gy, gz = g

            # ---- direct binning: q = (g - coff) / dx ; k = floor(q) ----
            qx = uu; qy = vv; qz = p0[0]        # reuse buffers
            invx = float(np.float32(1.0) / DXV)
            invz = float(np.float32(1.0) / DZV)
            nc.vector.tensor_scalar(out=qx[:], in0=gx[:], scalar1=float(COFFX),
                                    scalar2=invx, op0=AL.subtract, op1=AL.mult)
            nc.vector.tensor_scalar(out=qy[:], in0=gy[:], scalar1=float(COFFX),
                                    scalar2=invx, op0=AL.subtract, op1=AL.mult)
            nc.vector.tensor_scalar(out=qz[:], in0=gz[:], scalar1=float(COFFZ),
                                    scalar2=invz, op0=AL.subtract, op1=AL.mult)
            kxt = p0[1]; kyt = p0[2]
            # round-to-nearest then subtract (q < r) -> floor
            nc.vector.tensor_scalar(out=kxt[:], in0=qx[:], scalar1=float(MAGIC),
                                    scalar2=float(MAGIC), op0=AL.add, op1=AL.subtract)
            nc.vector.tensor_tensor(out=tmpa[:], in0=qx[:], in1=kxt[:], op=AL.is_lt)
            nc.vector.tensor_tensor(out=kxt[:], in0=kxt[:], in1=tmpa[:], op=AL.subtract)
            nc.vector.tensor_scalar(out=kyt[:], in0=qy[:], scalar1=float(MAGIC),
                                    scalar2=float(MAGIC), op0=AL.add, op1=AL.subtract)
            nc.vector.tensor_tensor(out=tmpa[:], in0=qy[:], in1=kyt[:], op=AL.is_lt)
            nc.vector.tensor_tensor(out=kyt[:], in0=kyt[:], in1=tmpa[:], op=AL.subtract)

            # ---- penalties: window containment + z in (-1, 1) ----
            pen = gx      # reuse
            nc.vector.tensor_tensor(out=g3(pen[:]), in0=g3(kxt[:]), in1=mslice(1), op=AL.is_ge)
            nc.vector.tensor_tensor(out=g3(tmpa[:]), in0=g3(kxt[:]), in1=mslice(2), op=AL.is_lt)
            nc.vector.tensor_tensor(out=pen[:], in0=pen[:], in1=tmpa[:], op=AL.mult)
            nc.vector.tensor_tensor(out=g3(tmpa[:]), in0=g3(kyt[:]), in1=mslice(3), op=AL.is_ge)
            nc.vector.tensor_tensor(out=pen[:], in0=pen[:], in1=tmpa[:], op=AL.mult)
            nc.vector.tensor_tensor(out=g3(tmpa[:]), in0=g3(kyt[:]), in1=mslice(4), op=AL.is_lt)
            nc.vector.tensor_tensor(out=pen[:], in0=pen[:], in1=tmpa[:], op=AL.mult)
            nc.vector.tensor_scalar(out=tmpa[:], in0=qz[:], scalar1=-1.0, scalar2=None, op0=AL.is_gt)
            nc.vector.tensor_tensor(out=pen[:], in0=pen[:], in1=tmpa[:], op=AL.mult)
            nc.vector.tensor_scalar(out=tmpa[:], in0=qz[:], scalar1=1.0, scalar2=None, op0=AL.is_lt)
            nc.vector.tensor_tensor(out=pen[:], in0=pen[:], in1=tmpa[:], op=AL.mult)

            # ---- stripe-local index: kyt*Rx + kxt - D0, clamp, apply penalty ----
            lidx = gy     # reuse
            nc.vector.tensor_scalar(out=lidx[:], in0=kyt[:], scalar1=float(Rx),
                                    scalar2=None, op0=AL.mult)
            nc.vector.tensor_tensor(out=lidx[:], in0=lidx[:], in1=kxt[:], op=AL.add)
            nc.vector.tensor_tensor(out=g3(lidx[:]), in0=g3(lidx[:]), in1=mslice(0), op=AL.subtract)
            nc.vector.tensor_scalar(out=lidx[:], in0=lidx[:], scalar1=-1000.0,
                                    scalar2=40000.0, op0=AL.max, op1=AL.min)
            nc.vector.tensor_scalar(out=tmpa[:], in0=pen[:], scalar1=-BIGPEN,
                                    scalar2=BIGPEN, op0=AL.mult, op1=AL.add)
            nc.vector.tensor_tensor(out=lidxT[:], in0=lidx[:], in1=tmpa[:], op=AL.add)

            region2d = region[:].rearrange("p (y x) -> p y x", x=Rx)
            region2db = regionb[:].rearrange("p (y x) -> p y x", x=Rx)
            DVE_JS = tuple(range(0, 5))
            POOL_JS = tuple(range(5, UJ))

            # ---- main unit loop ----
            def body(iv):
                cur = wp.tile([128, UJ], f32, tag="cur")
                nc.sync.dma_start(cur[:], lidxT[:, dslice(iv * UJ, UJ)])
                fb = wp.tile([128, UJ * C], f16, tag="fb")
                nc.sync.dma_start(fb[:], feats_t[dslice(iv, 1)].rearrange("o p x -> p (o x)"))
                oh = op_.tile([128, UJ * W], f16, tag="oh")
                for j in DVE_JS:
                    nc.vector.tensor_scalar(out=oh[:, j * W:(j + 1) * W], in0=iotaf[:],
                                            scalar1=cur[:, j:j + 1], scalar2=None,
                                            op0=AL.is_equal)
                for j in POOL_JS:
                    nc.gpsimd.tensor_scalar(out=oh[:, j * W:(j + 1) * W], in0=iotaf[:],
                                            scalar1=cur[:, j:j + 1], scalar2=None,
                                            op0=AL.is_equal)
                ps = pp.tile([C, W], f32, space="PSUM", tag="ps")
                for j in range(UJ):
                    for (a, bnd) in bank_slices:
                        nc.tensor.matmul(ps[:, a:bnd], lhsT=fb[:, j * C:(j + 1) * C],
                                         rhs=oh[:, j * W + a:j * W + bnd],
                                         start=(j == 0), stop=(j == UJ - 1))
                pss = wp.tile([C, W], f16, tag="pss")
                nc.scalar.copy(out=pss[:], in_=ps[:])
                nc.sync.dma_start(stag[:, dslice(iv * W, W)], pss[:])

            tc.For_i_unrolled(0, smax, 1, body, max_unroll=3)

            # ---- paste: python-unrolled, two parallel chains (DVE/Pool) ----
            slot_so = []
            off = 0
            for so, cnt in zip(plan['classes'], plan['cls_cnt']):
                slot_so += [so] * cnt
                off += cnt
            assert len(slot_so) == smax
            for s, so in enumerate(slot_so):
                src = stag[:, s * W:(s + 1) * W].rearrange("p (y x) -> p y x", x=Rx)
                if s % 2 == 0:
                    dst = region2d[:, so:so + H, :]
                    nc.vector.tensor_tensor(out=dst, in0=dst, in1=src, op=AL.add)
                else:
                    dst = region2db[:, so:so + H, :]
                    nc.gpsimd.tensor_tensor(out=dst, in0=dst, in1=src, op=AL.add)
            nc.vector.tensor_tensor(out=region[:], in0=region[:], in1=regionb[:], op=AL.add)

            # ---- epilogue: allreduce the region ----
            rpart = dp.tile([C, rcells], f32)
            rsum = dp.tile([C, rcells], f32)
            nc.sync.dma_start(rpart[:], region[:])
            nc.gpsimd.collective_compute(
                "AllReduce", AL.add,
                replica_groups=[list(range(NCORES))],
                ins=[rpart[:]], outs=[rsum[:]])
            nc.sync.dma_start(rout_t[:], rsum[:])

    nc.compile()
    return nc


def kernel(**inputs) -> np.ndarray:
    from concourse.bass_utils import run_bass_kernel_spmd

    plan = _build_plan(inputs)
    key = (plan['smax'], plan['W'], plan['H'], plan['Rx'], plan['Ry'],
           tuple(plan['classes']), tuple(plan['cls_cnt']))
    if key not in _CACHE:
        _CACHE.clear()
        _CACHE[key] = _build_bass(plan)
    nc = _CACHE[key]

    feats = _pack_feats(inputs['cam_feats'], plan)
    in_maps = []
    for k in range(NCORES):
        cpl = plan['cores'][k]
        in_maps.append(dict(feats=feats[k], pxt=plan['pxt'], pyt=cpl['pyt'],
                            coef=cpl['coef'], meta=cpl['meta']))
    r = run_bass_kernel_spmd(nc, in_maps, core_ids=list(range(NCORES)))
    region = r.results[0]['region_out']          # [C, rcells] summed over cores
    out = np.zeros((B, C, NX, NY), np.float32)
    Rx, Ry = plan['Rx'], plan['Ry']
    blk = region.reshape(C, Ry, Rx).transpose(0, 2, 1)
    out[0, :, plan['rx0']:plan['rx0'] + Rx, plan['ry0']:plan['ry0'] + Ry] = blk
    return out
